# revision 11
# baseline (speedup 1.0000x reference)
"""Trainium2 Bass kernel for nn_BilinearGrounding.

Reference computation:
    encI_p[b]  = encI[b] @ K_w.T + K_b                  # [100, 768]
    logits[b]  = encT[b] @ bil_w[0] @ encI_p[b].T       # [128, 100]
                 + bil_b[0] + mask[b, 0]

Kernel strategy:
  * One-time weight fold on host (deployment-style constant folding):
        M = bil_w[0] @ K_w    [768, 2048]
        c = bil_w[0] @ K_b    [768]
    so the device computes, per batch b:
        Y[b]      = M @ encI[b].T + c[:, None]          # [768, 100]
        logits[b] = encT[b] @ Y[b] + bil_b + mask[b]
  * Data-parallel over batch: 8 batches per core x 8 NeuronCores. Host
    supplies each core transposed, partition-chunked fp32 layouts so every
    matmul contraction dim sits on SBUF partitions (the PE reduces over
    partitions); no on-device transposes needed.
  * fp32 on the wire (HBM reads stay full precision); cast to bf16 during
    the SWDGE DMA; all matmuls bf16 with fp32 PSUM accumulation.
  * M^T and encI^T are concatenated host-side into one [2048, 1568] DRAM
    tensor loaded by 4 chunked DMAs; cvec+mask+bil_b are packed into one
    small tensor. This keeps the number of DMA sem lanes (and so the
    kernel-tail drain's sync-wait count) under the walrus per-instruction
    limit.
"""

import numpy as np

B, N_TOK, N_ROI = 64, 128, 100
T_HID, I_HID = 768, 2048
NCORES = 8
NB = B // NCORES          # batches per core
NCOL = NB * N_ROI         # 800  (stacked roi columns)
NTCOL = NB * N_TOK        # 1024 (stacked token columns)
IC = I_HID // 128         # 16 i-chunks (contraction for Y)
DC = T_HID // 128         # 6  d-chunks (contraction for logits)
WCOL = T_HID + NCOL       # 1568 combined M^T | encI^T columns
SMW = DC + NB * N_ROI     # 806 packed smalls columns (cvec | mask)

_CACHE = {}


def _build():
    import concourse.tile as tile
    from concourse import bacc, mybir
    from contextlib import ExitStack

    f32 = mybir.dt.float32
    bf16 = mybir.dt.bfloat16

    # Bacc (not plain Bass): its finalize() lowers multi-wait sync_info into
    # EVSEM chains — TRN2 instructions allow only one sync wait each.
    nc = bacc.Bacc("TRN2", target_bir_lowering=False)
    # big[i, 0:768] = M^T, big[i, 768:1568] = encI^T (cols b*100+r)
    d_big = nc.dram_tensor("big", [I_HID, WCOL], f32, kind="ExternalInput")
    d_enct = nc.dram_tensor("enct_t", [T_HID, NTCOL], f32, kind="ExternalInput")
    # sm[p, 0:6] = c chunks; sm[p, 6:806] = mask[b*128+p? -> see host] + bil_b
    d_sm = nc.dram_tensor("sm", [128, SMW], f32, kind="ExternalInput")
    d_out = nc.dram_tensor("out", [NTCOL, N_ROI], f32, kind="ExternalOutput")

    big_r = d_big[:, :].rearrange("(ic p) n -> p ic n", p=128)    # [128,16,1568]
    enct_r = d_enct[:, :].rearrange("(dc p) n -> p dc n", p=128)  # [128,6,1024]
    out_r = d_out[:, :].rearrange("(b p) r -> p b r", p=128)      # [128,8,100]

    with tile.TileContext(nc) as tc, ExitStack() as ctx:
        sb = ctx.enter_context(tc.tile_pool(name="sb", bufs=1))
        ps = ctx.enter_context(tc.tile_pool(name="ps", bufs=1, space="PSUM"))

        BIG = sb.tile([128, IC, WCOL], bf16)      # M^T | encI^T chunks
        ENCT = sb.tile([128, DC, NTCOL], bf16)    # encT^T chunks (lhsT)
        SM = sb.tile([128, SMW], f32)             # cvec | mask(+bil_b)
        Y = sb.tile([128, DC, NCOL], bf16)        # Y = M @ encI^T + c
        OUT = sb.tile([128, NB, N_ROI], f32)

        # ---- loads ----
        # 4 chunked casts (fp32 -> bf16 => SWDGE) so stage-Y matmuls start
        # as soon as the first i-chunks land.
        for j in range(4):
            sl = slice(4 * j, 4 * j + 4)
            nc.gpsimd.dma_start(out=BIG[:, sl, :], in_=big_r[:, sl, :])
        nc.gpsimd.dma_start(out=ENCT[:, :, :], in_=enct_r)
        nc.sync.dma_start(out=SM[:, :], in_=d_sm[:, :])

        # Warm the ACT/DVE vector clocks on the smalls DMA so downstream
        # consumers need only a single sync wait (walrus per-instruction
        # sync-slot limit).
        CVW = sb.tile([128, 1], f32, name="cvw")
        nc.scalar.copy(out=CVW[:, :], in_=SM[:, 0:1])
        MW = sb.tile([128, 1], f32, name="mw")
        nc.vector.tensor_copy(out=MW[:, :], in_=SM[:, 1:2])

        # ---- stage Y: Y[dc] = sum_ic MT[ic,dc].T @ ENCI[ic]  (+ c) ----
        # i-chunk-outer order tracks the DMA stream; two halves of 3 d-chunks
        # keep PSUM usage at 3 x [128, 800] f32 = 6 banks.
        for half in range(2):
            dcs = list(range(3 * half, 3 * half + 3))
            acc = {
                dc: ps.tile([128, NCOL], f32, tag=f"acc{dc % 3}", name=f"acc_{dc}")
                for dc in dcs
            }
            for ic in range(IC):
                for dc in dcs:
                    w = BIG[:, ic, dc * 128:(dc + 1) * 128]
                    # PSUM bank is 2KB => split N=800 into 512 + 288
                    nc.tensor.matmul(
                        acc[dc][:, 0:512], w,
                        BIG[:, ic, T_HID:T_HID + 512],
                        start=(ic == 0), stop=(ic == IC - 1))
                    nc.tensor.matmul(
                        acc[dc][:, 512:NCOL], w,
                        BIG[:, ic, T_HID + 512:T_HID + NCOL],
                        start=(ic == 0), stop=(ic == IC - 1))
            for dc in dcs:
                # PSUM -> SBUF copy with per-partition bias c, cast to bf16.
                # One copy per PSUM bank keeps sync waits per instruction low.
                nc.scalar.activation(
                    out=Y[:, dc, 0:512], in_=acc[dc][:, 0:512],
                    func=mybir.ActivationFunctionType.Identity,
                    bias=SM[:, dc:dc + 1])
                nc.scalar.activation(
                    out=Y[:, dc, 512:NCOL], in_=acc[dc][:, 512:NCOL],
                    func=mybir.ActivationFunctionType.Identity,
                    bias=SM[:, dc:dc + 1])

        # ---- stage logits: logits[b] = sum_dc ENCT[dc,b].T @ Y[dc,b] ----
        for b in range(NB):
            pc = ps.tile([128, N_ROI], f32, tag="psc", bufs=2, name=f"pc_{b}")
            for dc in range(DC):
                nc.tensor.matmul(
                    pc[:, :],
                    ENCT[:, dc, b * 128:(b + 1) * 128],
                    Y[:, dc, b * N_ROI:(b + 1) * N_ROI],
                    start=(dc == 0), stop=(dc == DC - 1))
            # out = psum + (mask + bil_b)  in one DVE op
            nc.vector.tensor_add(
                OUT[:, b, :], pc[:, :],
                SM[:, DC + b * N_ROI:DC + (b + 1) * N_ROI])
        nc.sync.dma_start(out=out_r, in_=OUT[:, :, :])

    # Run the Bacc passes (register allocation, EVSEM wait-splitting, ...);
    # the pjrt execution path serializes nc as-is without finalizing.
    nc.finalize()
    return nc


def _get_nc():
    if "nc" not in _CACHE:
        _CACHE["nc"] = _build()
    return _CACHE["nc"]


def _prep_in_maps(encT, encI, mask, K_w, K_b, bil_w, bil_b):
    encT = np.asarray(encT, np.float32)
    encI = np.asarray(encI, np.float32)
    mask = np.asarray(mask, np.float32)
    K_w = np.asarray(K_w, np.float32)
    K_b = np.asarray(K_b, np.float32)
    bil_w = np.asarray(bil_w, np.float32)
    bil_b = np.asarray(bil_b, np.float32)

    # One-time weight fold (f64 for accuracy)
    M = bil_w[0].astype(np.float64) @ K_w.astype(np.float64)
    c = bil_w[0].astype(np.float64) @ K_b.astype(np.float64)
    mt = M.T.astype(np.float32)                                   # [2048, 768]
    cvec = c.astype(np.float32).reshape(DC, 128).T                # [128, 6]

    in_maps = []
    for cid in range(NCORES):
        sl = slice(cid * NB, (cid + 1) * NB)
        enci_t = encI[sl].transpose(2, 0, 1).reshape(I_HID, NCOL)
        big = np.ascontiguousarray(np.concatenate([mt, enci_t], axis=1))
        enct_t = np.ascontiguousarray(
            encT[sl].transpose(2, 0, 1).reshape(T_HID, NTCOL))
        # mask packed as [p, b, r] with tok on partitions; bil_b folded in
        mask_p = (mask[sl, 0].transpose(1, 0, 2).reshape(128, NB * N_ROI)
                  + np.float32(bil_b[0]))
        sm = np.ascontiguousarray(
            np.concatenate([cvec, mask_p.astype(np.float32)], axis=1))
        in_maps.append({"big": big, "enct_t": enct_t, "sm": sm})
    return in_maps


def _run(inputs: dict, trace: bool = False):
    from concourse.bass_utils import run_bass_kernel_spmd

    in_maps = _prep_in_maps(**inputs)
    nc = _get_nc()
    res = run_bass_kernel_spmd(nc, in_maps, list(range(NCORES)), trace=trace)
    out = np.concatenate(
        [res.results[i]["out"].reshape(NB, N_TOK, N_ROI) for i in range(NCORES)],
        axis=0)
    return out, res


def kernel(**inputs) -> np.ndarray:
    out, _ = _run(inputs, trace=False)
    return out
